# revision 26
# baseline (speedup 1.0000x reference)
"""AntiSymmetric GNN (2x AntiSymmetricConv + linear layers + log_softmax)
distributed Bass kernel for 8 TRN2 NeuronCores.

Strategy:
  - Nodes sharded by destination across 8 cores (12500/core, padded 12544).
  - Edges partitioned by destination core; per core sorted by
    (source-chunk, dest-window); aggregation = dma_gather of pre-scaled
    source features (dinv[src] * (h @ phi.T)) from an AllGathered table,
    then one-hot scatter matmuls accumulating per 128-dest window.
  - gcn norm factorizes: agg[c] = dinv[c] * (sum_e T[src_e] + T[c]),
    T = dinv*xw; the self-loop term T[c] is added from the local table.
  - Tables are AllGathered in 4 window-aligned chunks (<=25600 rows so
    gather indices fit int16), pipelined with compute.
  - Per-core gather lengths differ; the SPMD graph uses the max length
    per (window, chunk) segment while each core passes its exact valid
    count in a register (tail indices are -1 = skipped by the DMA).
"""

import numpy as np
import ml_dtypes

N = 100_000
F_IN = 256
HID = 128
C = 32
EPS = 0.1
GAMMA = 0.1

NCORES = 8
SHARD = 12_500
PADN = 12_544            # 98 * 128
W = 98                   # windows per core
QW = [25, 25, 24, 24]    # windows per quarter (window-aligned chunks)
QROWS = [3200, 3200, 3072, 3072]
QSTART = [0, 3200, 6400, 9472]
QWSTART = [0, 25, 50, 74]
MAX_SEG = 1024           # SWDGE ring cap per dma_gather call

_CACHE = {}


def _host_prep(x, lin1_w, lin1_b, lin2_w, lin2_b, W1, phi1_w, b1, W2, phi2_w, b2,
               edge_index):
    bf16 = ml_dtypes.bfloat16
    row = edge_index[0].astype(np.int64)
    col = edge_index[1].astype(np.int64)

    # degrees INCLUDE self loops (reference appends them)
    deg = (np.bincount(col, minlength=N) + 1).astype(np.float32)
    dinv = 1.0 / np.sqrt(deg)

    # source -> (chunk, int16 index into chunk table)
    ks = row // SHARD
    i_s = row % SHARD
    wloc = i_s // 128
    q_s = np.where(wloc < 25, 0, np.where(wloc < 50, 1, np.where(wloc < 74, 2, 3)))
    pos = i_s - np.asarray(QSTART)[q_s]
    idx16_all = ks * np.asarray(QROWS)[q_s] + pos

    k_dst = col // SHARD

    cores = []
    L = np.zeros((NCORES, 4 * W), np.int64)
    for k in range(NCORES):
        m = k_dst == k
        r_idx = idx16_all[m]
        c_loc = col[m] - k * SHARD
        key = q_s[m] * W + c_loc // 128
        order = np.argsort(key, kind="stable")
        cores.append((key[order], r_idx[order],
                      (c_loc % 128)[order].astype(np.float32)))
        L[k] = np.bincount(cores[k][0], minlength=4 * W)

    LMAX = np.maximum(L.max(axis=0), (L.max(axis=0) > 0).astype(np.int64))
    assert LMAX.max() <= MAX_SEG, f"segment too large: {LMAX.max()}"

    cols16 = (LMAX + 15) // 16
    tiles = (LMAX + 127) // 128
    seg_off16 = np.zeros(4 * W + 1, np.int64)
    np.cumsum(cols16, out=seg_off16[1:])
    seg_offt = np.zeros(4 * W + 1, np.int64)
    np.cumsum(tiles, out=seg_offt[1:])

    per_core = []
    for k in range(NCORES):
        key_s, idx_s, cl_s = cores[k]
        idx_arr = np.full(int(seg_off16[-1]) * 16, -1, np.int16)
        colv_arr = np.full(int(seg_offt[-1]) * 128, -1.0, np.float32)
        starts_src = np.zeros(4 * W + 1, np.int64)
        np.cumsum(L[k], out=starts_src[1:])
        seg_of = np.repeat(np.arange(4 * W), L[k])
        off_in = np.arange(len(idx_s)) - starts_src[seg_of]
        idx_arr[seg_off16[seg_of] * 16 + off_in] = idx_s.astype(np.int16)
        colv_arr[seg_offt[seg_of] * 128 + off_in] = cl_s
        # pad [L_k, lmax) of each segment with cycled copies of its own real
        # indices (S zeroes them) -- spreads pad reads across HBM instead of
        # hammering row 0
        for sgi in np.nonzero(LMAX > L[k])[0]:
            b = int(seg_off16[sgi]) * 16
            lk = int(L[k][sgi])
            lm = int(LMAX[sgi])
            if lk > 0:
                idx_arr[b + lk:b + lm] = np.resize(idx_arr[b:b + lk], lm - lk)
            else:
                idx_arr[b + lk:b + lm] = 0
        per_core.append((idx_arr, colv_arr))

    def wrap_idx(arr):
        a16 = arr.reshape(-1, 16).T
        return np.ascontiguousarray(np.tile(a16, (8, 1)))

    in_maps = []
    for k in range(NCORES):
        idx_arr, colv_arr = per_core[k]
        xs = np.zeros((PADN, F_IN), np.float32)
        xs[:SHARD] = x[k * SHARD:(k + 1) * SHARD]
        xT = np.ascontiguousarray(xs.T).astype(bf16)
        dvk = np.zeros(PADN, np.float32)
        dvk[:SHARD] = dinv[k * SHARD:(k + 1) * SHARD]
        im = {
            "xT": xT,
            "dinv_w": np.ascontiguousarray(dvk.reshape(W, 128).T),
            "lin1T": np.ascontiguousarray(lin1_w.T).astype(bf16),
            "phi1T": np.ascontiguousarray(phi1_w.T).astype(bf16),
            "aw1T": np.ascontiguousarray(
                (W1 - W1.T - GAMMA * np.eye(HID, dtype=np.float32)).T).astype(bf16),
            "lin2T": np.ascontiguousarray(lin2_w.T).astype(bf16),
            "phi2T": np.ascontiguousarray(phi2_w.T).astype(bf16),
            "aw2T": np.ascontiguousarray(
                (W2 - W2.T - GAMMA * np.eye(C, dtype=np.float32)).T).astype(bf16),
        }
        for q in range(4):
            s16 = slice(int(seg_off16[q * W]) * 16, int(seg_off16[(q + 1) * W]) * 16)
            st = slice(int(seg_offt[q * W]) * 128, int(seg_offt[(q + 1) * W]) * 128)
            im[f"idx{q}"] = wrap_idx(idx_arr[s16])
            im[f"colv{q}"] = np.ascontiguousarray(
                colv_arr[st].reshape(-1, 128).T.astype(bf16))
        in_maps.append(im)

    biases = {
        "blin1": np.broadcast_to(lin1_b, (128, HID)).astype(np.float32).copy(),
        "bconv1": np.broadcast_to(b1, (128, HID)).astype(np.float32).copy(),
        "blin2": np.broadcast_to(lin2_b, (128, C)).astype(np.float32).copy(),
        "bconv2": np.broadcast_to(b2, (128, C)).astype(np.float32).copy(),
    }
    use_bias = {name: bool(np.any(arr)) for name, arr in biases.items()}
    for name, used in use_bias.items():
        if used:
            for im in in_maps:
                im[name] = biases[name]

    import os
    meta = {
        "LMAX": LMAX,
        "tiles": tiles, "cols16": cols16,
        "seg_off16": seg_off16, "seg_offt": seg_offt,
        "use_bias": use_bias,
        "phases": int(os.environ.get("KERNEL_PHASES", "5")),
    }
    return in_maps, meta


def _build_graph(meta):
    import concourse.bass as bass
    import concourse.mybir as mybir
    import concourse.tile as tile
    from concourse import bacc
    from concourse.masks import make_identity
    from contextlib import ExitStack

    dt = mybir.dt
    Alu = mybir.AluOpType
    Act = mybir.ActivationFunctionType
    LMAX = meta["LMAX"]
    tiles = meta["tiles"]
    cols16 = meta["cols16"]
    seg_off16 = meta["seg_off16"]
    seg_offt = meta["seg_offt"]
    use_bias = meta["use_bias"]
    phases = meta.get("phases", 5)

    nc = bacc.Bacc("TRN2", target_bir_lowering=False, num_swdge_queues=4)

    xT = nc.declare_dram_parameter("xT", [F_IN, PADN], dt.bfloat16, isOutput=False)
    dinv_w = nc.declare_dram_parameter("dinv_w", [128, W], dt.float32, isOutput=False)
    lin1T = nc.declare_dram_parameter("lin1T", [F_IN, HID], dt.bfloat16, isOutput=False)
    phi1T = nc.declare_dram_parameter("phi1T", [HID, HID], dt.bfloat16, isOutput=False)
    aw1T = nc.declare_dram_parameter("aw1T", [HID, HID], dt.bfloat16, isOutput=False)
    lin2T = nc.declare_dram_parameter("lin2T", [HID, C], dt.bfloat16, isOutput=False)
    phi2T = nc.declare_dram_parameter("phi2T", [C, C], dt.bfloat16, isOutput=False)
    aw2T = nc.declare_dram_parameter("aw2T", [C, C], dt.bfloat16, isOutput=False)
    idx_p, colv_p = [], []
    for q in range(4):
        n16 = int(seg_off16[(q + 1) * W] - seg_off16[q * W])
        nt = int(seg_offt[(q + 1) * W] - seg_offt[q * W])
        idx_p.append(nc.declare_dram_parameter(f"idx{q}", [128, n16], dt.int16,
                                               isOutput=False))
        colv_p.append(nc.declare_dram_parameter(f"colv{q}", [128, nt],
                                                dt.bfloat16, isOutput=False))
    bias_p = {}
    for name, shape in [("blin1", [128, HID]), ("bconv1", [128, HID]),
                        ("blin2", [128, C]), ("bconv2", [128, C])]:
        if use_bias[name]:
            bias_p[name] = nc.declare_dram_parameter(name, shape, dt.float32,
                                                     isOutput=False)
    out_p = nc.declare_dram_parameter("out", [PADN, C], dt.float32, isOutput=True)

    t1q_in = [nc.dram_tensor(f"t1in{q}", [QROWS[q], HID], dt.bfloat16)
              for q in range(4)]
    t1q_tab = [nc.dram_tensor(f"t1tab{q}", [NCORES * QROWS[q], HID], dt.bfloat16,
                              addr_space="Shared") for q in range(4)]
    t2q_in = [nc.dram_tensor(f"t2in{q}", [QROWS[q], C], dt.float32)
              for q in range(4)]
    t2q_pack = [nc.dram_tensor(f"t2pk{q}", [NCORES * QROWS[q], C], dt.float32,
                               addr_space="Shared") for q in range(4)]
    t2q_tab = [nc.dram_tensor(f"t2tab{q}", [NCORES * QROWS[q], 64], dt.float32)
               for q in range(4)]

    rg = [list(range(NCORES))]

    with tile.TileContext(nc) as tc, ExitStack() as top:
        const = top.enter_context(tc.tile_pool(name="const", bufs=1))
        h2_pool = top.enter_context(tc.tile_pool(name="h2", bufs=1))
        agg2_pool = top.enter_context(tc.tile_pool(name="agg2", bufs=1))
        tmp_pool = top.enter_context(tc.tile_pool(name="tmp", bufs=3))

        lin1T_sb = const.tile([128, 2, HID], dt.bfloat16)
        nc.sync.dma_start(lin1T_sb[:], lin1T[:].rearrange("(t p) j -> p t j", p=128))
        phi1T_sb = const.tile([128, HID], dt.bfloat16)
        nc.sync.dma_start(phi1T_sb[:], phi1T[:])
        aw1T_sb = const.tile([128, HID], dt.bfloat16)
        nc.sync.dma_start(aw1T_sb[:], aw1T[:])
        lin2T_sb = const.tile([128, C], dt.bfloat16)
        nc.sync.dma_start(lin2T_sb[:], lin2T[:])
        phi2T_sb = const.tile([128, C], dt.bfloat16)
        aw2T_sb = const.tile([128, C], dt.bfloat16)
        for r in range(4):
            nc.sync.dma_start(phi2T_sb[r * C:(r + 1) * C, :], phi2T[:])
            nc.sync.dma_start(aw2T_sb[r * C:(r + 1) * C, :], aw2T[:])
        dinv_sb = const.tile([128, W], dt.float32)
        nc.sync.dma_start(dinv_sb[:], dinv_w[:])
        bias_sb = {}
        for name, p in bias_p.items():
            t = const.tile(list(p.shape), dt.float32)
            nc.sync.dma_start(t[:], p[:])
            bias_sb[name] = t

        iota_i = const.tile([128, 128], dt.int32)
        nc.gpsimd.iota(iota_i[:], pattern=[[1, 128]], base=0, channel_multiplier=0)
        iota_bf = const.tile([128, 128], dt.bfloat16)
        nc.vector.tensor_copy(iota_bf[:], iota_i[:])
        ident = const.tile([128, 128], dt.bfloat16)
        make_identity(nc, ident[:])

        h2 = h2_pool.tile([128, W, C], dt.float32)
        agg2 = agg2_pool.tile([128, W, C], dt.float32)
        h2T_all = h2_pool.tile([128, (W + 1) // 2, 128], dt.bfloat16, tag="h2T_all")

        MAXTILES = int(tiles.max()) if len(tiles) else 1

        def aggregate(table, fw, elem, pools, q, post_cb=None):
            gp, sp, bp, ip, cp, psX, agg_t = pools
            n16_0 = int(seg_off16[q * W])
            nt_0 = int(seg_offt[q * W])
            n16 = int(seg_off16[(q + 1) * W]) - n16_0
            ntq = int(seg_offt[(q + 1) * W]) - nt_0
            idx_sb = ip.tile([128, n16], dt.int16, tag="idx")
            nc.sync.dma_start(idx_sb[:], idx_p[q][:])
            colv_sb = cp.tile([128, ntq], dt.bfloat16, tag="colv")
            nc.sync.dma_start(colv_sb[:], colv_p[q][:])
            for w in range(W):
                s = q * W + w
                nt = int(tiles[s])
                if nt == 0:
                    if post_cb is not None:
                        post_cb(w)
                    continue
                lmax = int(LMAX[s])
                o16 = int(seg_off16[s]) - n16_0
                ot = int(seg_offt[s]) - nt_0
                if fw == HID:
                    g = gp.tile([128, nt, HID], dt.bfloat16, tag="g")
                    rhs = g
                else:
                    g = gp.tile([128, nt, 64], dt.float32, tag="g")
                nc.gpsimd.dma_gather(
                    g[:], table[q][:], idx_sb[:, o16:o16 + int(cols16[s])],
                    lmax, lmax, elem, queue_num=w % 4)
                if fw != HID:
                    rhs = bp.tile([128, nt, C], dt.bfloat16, tag="gb")
                    nc.vector.tensor_copy(rhs[:], g[:, :, 0:C])
                S = sp.tile([128, nt, 128], dt.bfloat16, tag="S")
                nc.vector.tensor_tensor(
                    S[:],
                    iota_bf[:].unsqueeze(1).broadcast_to([128, nt, 128]),
                    colv_sb[:, ot:ot + nt].unsqueeze(2)
                        .broadcast_to([128, nt, 128]),
                    op=Alu.is_equal)
                pseg = psX.tile([128, fw], dt.float32, tag="pseg")
                for j in range(nt):
                    nc.tensor.matmul(pseg[:], S[:, j, :], rhs[:, j, :],
                                     start=(j == 0), stop=(j == nt - 1))
                nc.vector.tensor_tensor(agg_t[:, w, :], agg_t[:, w, :], pseg[:],
                                        op=Alu.add)
                if post_cb is not None:
                    post_cb(w)

        with tc.tile_pool(name="h1cf", bufs=1) as h1_pool, \
             tc.tile_pool(name="agg", bufs=1) as agg_pool, ExitStack() as stA, \
             ExitStack() as stC, ExitStack() as stD, ExitStack() as stF:
            h1 = h1_pool.tile([128, W, HID], dt.bfloat16)
            h1T_all = h1_pool.tile([128, W, HID], dt.bfloat16, tag="h1T_all")
            agg = agg_pool.tile([128, W, HID], dt.float32)

            gp = stC.enter_context(tc.tile_pool(name="gp", bufs=8))
            sp = stC.enter_context(tc.tile_pool(name="sp", bufs=4))
            ip = stC.enter_context(tc.tile_pool(name="ip", bufs=2))
            cp = stC.enter_context(tc.tile_pool(name="cp", bufs=2))
            psC = stC.enter_context(tc.tile_pool(name="psC", bufs=4, space="PSUM"))
            xqp = stA.enter_context(tc.tile_pool(name="xq", bufs=1))
            t1qbp = stA.enter_context(tc.tile_pool(name="t1qb", bufs=2))
            psA = stA.enter_context(tc.tile_pool(name="psA", bufs=1, space="PSUM"))
            psAT = stA.enter_context(tc.tile_pool(name="psAT", bufs=1, space="PSUM"))

            nc.vector.memset(agg[:], 0.0)
            nc.vector.memset(h2[:], 0.0)
            nc.vector.memset(agg2[:], 0.0)
            for i in range(8):
                z = gp.tile([128, MAXTILES, HID], dt.bfloat16, tag="g")
                nc.vector.memset(z[:], 0.0)

            # -- phase A micro-steps (2-stage software pipeline) --
            astate = {}

            def a_s1(q, wi):
                w = QWSTART[q] + wi
                if wi == 0:
                    c0 = QWSTART[q] * 128
                    cw = QW[q] * 128
                    xq_sb = xqp.tile([128, 2, cw], dt.bfloat16, tag="xq")
                    nc.sync.dma_start(
                        xq_sb[:],
                        xT[:, c0:c0 + cw].rearrange("(t p) c -> p t c", p=128))
                    astate["xq"] = xq_sb
                    t1qb_t = t1qbp.tile([128, QW[q], HID], dt.bfloat16,
                                        tag="t1qb")
                    astate["t1qb"] = t1qb_t
                xq_sb = astate["xq"]
                ph = psA.tile([128, HID], dt.float32, tag="ph")
                nc.tensor.matmul(ph[:], xq_sb[:, 0, wi * 128:(wi + 1) * 128],
                                 lin1T_sb[:, 0, :], start=True, stop=False)
                nc.tensor.matmul(ph[:], xq_sb[:, 1, wi * 128:(wi + 1) * 128],
                                 lin1T_sb[:, 1, :], start=False, stop=True)
                if "blin1" in bias_sb:
                    t = tmp_pool.tile([128, HID], dt.float32, tag="tA")
                    nc.vector.tensor_tensor(t[:], ph[:], bias_sb["blin1"][:],
                                            op=Alu.add)
                    nc.scalar.activation(h1[:, w, :], t[:], Act.Relu)
                else:
                    nc.scalar.activation(h1[:, w, :], ph[:], Act.Relu)

            def a_s2(q, wi):
                w = QWSTART[q] + wi
                t1qb = astate["t1qb"]
                pt = psAT.tile([128, 128], dt.bfloat16, tag="pt")
                nc.tensor.transpose(pt[:], h1[:, w, :], ident[:])
                nc.scalar.copy(h1T_all[:, w, :], pt[:])
                pT = psA.tile([128, HID], dt.float32, tag="pT1")
                nc.tensor.matmul(pT[:], h1T_all[:, w, :], phi1T_sb[:],
                                 start=True, stop=True)
                nc.scalar.activation(t1qb[:, wi, :], pT[:], Act.Copy,
                                     scale=dinv_sb[:, w:w + 1])

            def a_tail(q):
                t1qb = astate["t1qb"]
                nc.vector.tensor_tensor(
                    agg[:, QWSTART[q]:QWSTART[q] + QW[q], :],
                    agg[:, QWSTART[q]:QWSTART[q] + QW[q], :], t1qb[:],
                    op=Alu.add)
                nc.sync.dma_start(
                    t1q_in[q][:].rearrange("(w p) f -> p w f", p=128), t1qb[:])
                nc.gpsimd.collective_compute(
                    "AllGather", Alu.bypass, replica_groups=rg,
                    ins=[t1q_in[q][:].opt()], outs=[t1q_tab[q][:].opt()])

            def sched_cb(stages, tail=None):
                """stages: (fn, count, start_w); fire fn(i) at w=start_w+2i.
                The tail fires right after the last stage item so cross-core
                collectives start (and finish) mid-pass, not at pass end."""
                sched = [[] for _ in range(W)]
                last_w = 0
                for fn, cnt, s0 in stages:
                    for i in range(cnt):
                        w = min(s0 + 2 * i, W - 1)
                        sched[w].append((fn, i))
                        last_w = max(last_w, w)
                if tail is not None:
                    sched[min(last_w + 1, W - 1)].append((tail, None))

                def cb(w):
                    for fn, i in sched[w]:
                        if i is None:
                            fn()
                        else:
                            fn(i)
                return cb

            # -- phase D micro-steps (3-stage software pipeline) --
            dstate = {}

            def d_s1(Q, i):
                w = QWSTART[Q] + i
                if i == 0:
                    t2qb_t = t2qbp.tile([128, QW[Q], C], dt.float32,
                                        tag="t2qb")
                    dstate["t2qb"] = t2qb_t
                paw = psD.tile([128, HID], dt.float32, tag="paw")
                nc.tensor.matmul(paw[:], h1T_all[:, w, :], aw1T_sb[:],
                                 start=True, stop=True)
                pre = tmp_pool.tile([128, HID], dt.float32, tag="pre")
                nc.vector.scalar_tensor_tensor(
                    pre[:], agg[:, w, :], dinv_sb[:, w:w + 1], paw[:],
                    op0=Alu.mult, op1=Alu.add)
                if "bconv1" in bias_sb:
                    nc.vector.tensor_tensor(pre[:], pre[:],
                                            bias_sb["bconv1"][:], op=Alu.add)
                th = tmp_pool.tile([128, HID], dt.float32, tag="th")
                nc.scalar.activation(th[:], pre[:], Act.Tanh)
                h1p = tmp_pool.tile([128, HID], dt.bfloat16, tag="h1p")
                nc.vector.scalar_tensor_tensor(
                    h1p[:], th[:], 0.1, h1[:, w, :], op0=Alu.mult, op1=Alu.add)
                dstate[("h1p", i)] = h1p

            def d_s2(Q, i):
                w = QWSTART[Q] + i
                h1p = dstate.pop(("h1p", i))
                pt2 = psDt.tile([128, 128], dt.bfloat16, tag="ptD")
                nc.tensor.transpose(pt2[:], h1p[:], ident[:])
                h1pT = tmp_pool.tile([128, 128], dt.bfloat16, tag="h1pT")
                nc.scalar.copy(h1pT[:], pt2[:])
                ph2 = psDs.tile([128, C], dt.float32, tag="psD2")
                nc.tensor.matmul(ph2[:], h1pT[:], lin2T_sb[:],
                                 start=True, stop=True)
                if "blin2" in bias_sb:
                    nc.vector.tensor_tensor(h2[:, w, :], ph2[:],
                                            bias_sb["blin2"][:], op=Alu.add)
                else:
                    nc.scalar.copy(h2[:, w, :], ph2[:])
                h2b = tmp_pool.tile([128, C], dt.bfloat16, tag="h2b")
                nc.vector.tensor_copy(h2b[:], h2[:, w, :])
                dstate[("h2b", i)] = h2b

            def d_s3(Q, i):
                w = QWSTART[Q] + i
                t2qb = dstate["t2qb"]
                h2b = dstate.pop(("h2b", i))
                pt3 = psDt.tile([C, 128], dt.bfloat16, tag="pt3")
                nc.tensor.transpose(pt3[:], h2b[:], ident[:])
                p0 = (w % 2) * 64
                nc.scalar.copy(h2T_all[p0:p0 + C, w // 2, :], pt3[:])
                pT2 = psDs.tile([128, C], dt.float32, tag="psD2")
                nc.tensor.matmul(pT2[:], h2T_all[p0:p0 + C, w // 2, :],
                                 phi2T_sb[p0:p0 + C, :], start=True, stop=True)
                nc.scalar.activation(t2qb[:, i, :], pT2[:], Act.Copy,
                                     scale=dinv_sb[:, w:w + 1])

            def d_tail(Q):
                t2qb = dstate["t2qb"]
                nc.vector.tensor_tensor(
                    agg2[:, QWSTART[Q]:QWSTART[Q] + QW[Q], :],
                    agg2[:, QWSTART[Q]:QWSTART[Q] + QW[Q], :], t2qb[:],
                    op=Alu.add)
                nc.sync.dma_start(
                    t2q_in[Q][:].rearrange("(w p) f -> p w f", p=128), t2qb[:])
                nc.gpsimd.collective_compute(
                    "AllGather", Alu.bypass, replica_groups=rg,
                    ins=[t2q_in[Q][:].opt()], outs=[t2q_pack[Q][:].opt()])
                nc.sync.dma_start(t2q_tab[Q][:, 0:C], t2q_pack[Q][:])

            def d_spread(Q):
                return sched_cb([(lambda i, Q=Q: d_s1(Q, i), QW[Q], 0),
                                 (lambda i, Q=Q: d_s2(Q, i), QW[Q], 2),
                                 (lambda i, Q=Q: d_s3(Q, i), QW[Q], 4)],
                                tail=lambda Q=Q: d_tail(Q))

            # ===== Phase A quarter 0 (bulk) =====
            for wi in range(QW[0]):
                a_s1(0, wi)
            for wi in range(QW[0]):
                a_s2(0, wi)
            a_tail(0)

            # ===== Phase C passes, carrying A(q+1) then D(0) =====
            if phases >= 2:
                for q in range(4):
                    if q < 3:
                        cb = sched_cb([(lambda i, Q=q + 1: a_s1(Q, i),
                                        QW[q + 1], 0),
                                       (lambda i, Q=q + 1: a_s2(Q, i),
                                        QW[q + 1], 2)],
                                      tail=lambda Q=q + 1: a_tail(Q))
                    else:
                        stA.close()
                        t2qbp = stD.enter_context(
                            tc.tile_pool(name="t2qb", bufs=2))
                        psD = stD.enter_context(
                            tc.tile_pool(name="psD", bufs=1, space="PSUM"))
                        psDt = stD.enter_context(
                            tc.tile_pool(name="psDt", bufs=1, space="PSUM"))
                        psDs = stD.enter_context(
                            tc.tile_pool(name="psDs", bufs=1, space="PSUM"))
                        cb = d_spread(0) if phases >= 3 else None
                    aggregate(t1q_tab, HID, HID,
                              (gp, sp, None, ip, cp, psC, agg), q, post_cb=cb)

            # ===== Phase F passes, carrying D(q+1) then G =====
            if phases >= 4:
                stD.close()
                stC.close()
                t2qbp = stF.enter_context(tc.tile_pool(name="t2qb2", bufs=2))
                psD = stF.enter_context(
                    tc.tile_pool(name="psD2p", bufs=1, space="PSUM"))
                psDt = stF.enter_context(
                    tc.tile_pool(name="psDt2p", bufs=1, space="PSUM"))
                psDs = stF.enter_context(
                    tc.tile_pool(name="psDs2p", bufs=1, space="PSUM"))
                gp2 = stF.enter_context(tc.tile_pool(name="gp2", bufs=8))
                sp2 = stF.enter_context(tc.tile_pool(name="sp2", bufs=4))
                bp2 = stF.enter_context(tc.tile_pool(name="bp2", bufs=4))
                ip2 = stF.enter_context(tc.tile_pool(name="ip2", bufs=2))
                cp2 = stF.enter_context(tc.tile_pool(name="cp2", bufs=2))
                psF = stF.enter_context(
                    tc.tile_pool(name="psF", bufs=2, space="PSUM"))
                psG = stF.enter_context(
                    tc.tile_pool(name="psG", bufs=2, space="PSUM"))
                for i in range(8):
                    z = gp2.tile([128, MAXTILES, 64], dt.float32, tag="g")
                    nc.vector.memset(z[:], 0.0)

                GB = 4
                import os as _os
                GFUSE = bool(int(_os.environ.get("KERNEL_GFUSE", "0")))

                def g_group(w0, gw):
                    a1 = tmp_pool.tile([128, GB, C], dt.float32, tag="a1g")
                    nc.vector.tensor_tensor(
                        a1[:, 0:gw, :], agg2[:, w0:w0 + gw, :],
                        dinv_sb[:, w0:w0 + gw].unsqueeze(2)
                            .broadcast_to([128, gw, C]),
                        op=Alu.mult)
                    pre = tmp_pool.tile([128, GB, C], dt.float32, tag="preg")
                    for wi in range(gw):
                        w = w0 + wi
                        p0 = (w % 2) * 64
                        pawt = psG.tile([128, C], dt.float32, tag="pawt")
                        nc.tensor.matmul(pawt[:],
                                         h2T_all[p0:p0 + C, w // 2, :],
                                         aw2T_sb[p0:p0 + C, :],
                                         start=True, stop=True)
                        nc.vector.tensor_tensor(
                            pre[:, wi, :], a1[:, wi, :], pawt[:], op=Alu.add)
                    if "bconv2" in bias_sb:
                        nc.vector.tensor_tensor(
                            pre[:, 0:gw, :], pre[:, 0:gw, :],
                            bias_sb["bconv2"][:].unsqueeze(1)
                                .broadcast_to([128, gw, C]),
                            op=Alu.add)
                    th = tmp_pool.tile([128, GB, C], dt.float32, tag="thg")
                    nc.scalar.activation(th[:, 0:gw, :], pre[:, 0:gw, :],
                                         Act.Tanh)
                    h2p = tmp_pool.tile([128, GB, C], dt.float32, tag="h2pg")
                    nc.vector.scalar_tensor_tensor(
                        h2p[:, 0:gw, :], th[:, 0:gw, :], 0.1,
                        h2[:, w0:w0 + gw, :], op0=Alu.mult, op1=Alu.add)
                    negmax = tmp_pool.tile([128, GB, 1], dt.float32, tag="nmg")
                    nc.vector.tensor_reduce(negmax[:, 0:gw, :], h2p[:, 0:gw, :],
                                            axis=mybir.AxisListType.X,
                                            op=Alu.max, negate=True)
                    sub = tmp_pool.tile([128, GB, C], dt.float32, tag="subg")
                    nc.vector.tensor_tensor(
                        sub[:, 0:gw, :], h2p[:, 0:gw, :],
                        negmax[:, 0:gw, :].broadcast_to([128, gw, C]),
                        op=Alu.add)
                    e = tmp_pool.tile([128, GB, C], dt.float32, tag="eg")
                    nc.scalar.activation(e[:, 0:gw, :], sub[:, 0:gw, :], Act.Exp)
                    ssum = tmp_pool.tile([128, GB, 1], dt.float32, tag="ssg")
                    nc.vector.tensor_reduce(ssum[:, 0:gw, :], e[:, 0:gw, :],
                                            axis=mybir.AxisListType.X,
                                            op=Alu.add)
                    lse = tmp_pool.tile([128, GB, 1], dt.float32, tag="lseg")
                    nc.scalar.activation(lse[:, 0:gw, :], ssum[:, 0:gw, :],
                                         Act.Ln)
                    nc.vector.tensor_tensor(
                        agg2[:, w0:w0 + gw, :], sub[:, 0:gw, :],
                        lse[:, 0:gw, :].broadcast_to([128, gw, C]),
                        op=Alu.subtract)

                def g_step(w):
                    if phases < 5:
                        return
                    if (w + 1) % GB == 0 or w == W - 1:
                        w0 = (w // GB) * GB
                        g_group(w0, w - w0 + 1)

                for q in range(4):
                    if q < 3:
                        cb = d_spread(q + 1)
                    else:
                        cb = g_step if GFUSE else None
                    aggregate(t2q_tab, C, 64,
                              (gp2, sp2, bp2, ip2, cp2, psF, agg2), q,
                              post_cb=cb)
                if not GFUSE and phases >= 5:
                    w0 = 0
                    while w0 < W:
                        gw = min(GB, W - w0)
                        g_group(w0, gw)
                        w0 += gw

        nc.sync.dma_start(out_p[:].rearrange("(w p) c -> p w c", p=128), agg2[:])

    nc.compile()
    return nc


def kernel(**inputs):
    from concourse.bass_utils import run_bass_kernel_spmd

    inp = {k: np.asarray(v) for k, v in inputs.items()}
    in_maps, meta = _host_prep(**inp)

    key = ("graph", tuple(meta["LMAX"].tolist()),
           tuple(sorted(meta["use_bias"].items())), meta["phases"])
    if key not in _CACHE:
        _CACHE[key] = _build_graph(meta)
    nc = _CACHE[key]

    import os
    trace = bool(int(os.environ.get("KERNEL_TRACE", "0")))
    res = run_bass_kernel_spmd(nc, in_maps, list(range(NCORES)), trace=trace,
                               tmpdir=os.environ.get("KERNEL_TRACE_DIR"))
    global LAST_EXEC_NS
    LAST_EXEC_NS = res.exec_time_ns

    out = np.concatenate([res.results[k]["out"][:SHARD] for k in range(NCORES)], 0)
    return out.astype(np.float32)


LAST_EXEC_NS = None



# revision 29
# speedup vs baseline: 1.3686x; 1.3686x over previous
"""AntiSymmetric GNN (2x AntiSymmetricConv + linear layers + log_softmax)
distributed Bass kernel for 8 TRN2 NeuronCores.

Strategy:
  - Nodes sharded by destination across 8 cores (12500/core, padded 12544).
  - Edges partitioned by destination core; per core sorted by
    (source-chunk, dest-window); aggregation = dma_gather of pre-scaled
    source features (dinv[src] * (h @ phi.T)) from an AllGathered table,
    then one-hot scatter matmuls accumulating per 128-dest window.
  - gcn norm factorizes: agg[c] = dinv[c] * (sum_e T[src_e] + T[c]),
    T = dinv*xw; the self-loop term T[c] is added from the local table.
  - Tables are AllGathered in 4 window-aligned chunks (<=25600 rows so
    gather indices fit int16), pipelined with compute.
  - Per-core gather lengths differ; the SPMD graph uses the max length
    per (window, chunk) segment while each core passes its exact valid
    count in a register (tail indices are -1 = skipped by the DMA).
"""

import numpy as np
import ml_dtypes

N = 100_000
F_IN = 256
HID = 128
C = 32
EPS = 0.1
GAMMA = 0.1

NCORES = 8
SHARD = 12_500
PADN = 12_544            # 98 * 128
W = 98                   # windows per core
QW = [25, 25, 24, 24]    # windows per quarter (window-aligned chunks)
QROWS = [3200, 3200, 3072, 3072]
QSTART = [0, 3200, 6400, 9472]
QWSTART = [0, 25, 50, 74]
MAX_SEG = 1024           # SWDGE ring cap per dma_gather call

_CACHE = {}


def _host_prep(x, lin1_w, lin1_b, lin2_w, lin2_b, W1, phi1_w, b1, W2, phi2_w, b2,
               edge_index):
    bf16 = ml_dtypes.bfloat16
    row = edge_index[0].astype(np.int64)
    col = edge_index[1].astype(np.int64)

    # degrees INCLUDE self loops (reference appends them)
    deg = (np.bincount(col, minlength=N) + 1).astype(np.float32)
    dinv = 1.0 / np.sqrt(deg)

    # source -> (chunk, int16 index into chunk table)
    ks = row // SHARD
    i_s = row % SHARD
    wloc = i_s // 128
    q_s = np.where(wloc < 25, 0, np.where(wloc < 50, 1, np.where(wloc < 74, 2, 3)))
    pos = i_s - np.asarray(QSTART)[q_s]
    idx16_all = ks * np.asarray(QROWS)[q_s] + pos

    k_dst = col // SHARD

    cores = []
    L = np.zeros((NCORES, 4 * W), np.int64)
    for k in range(NCORES):
        m = k_dst == k
        r_idx = idx16_all[m]
        c_loc = col[m] - k * SHARD
        key = q_s[m] * W + c_loc // 128
        order = np.argsort(key, kind="stable")
        cores.append((key[order], r_idx[order],
                      (c_loc % 128)[order].astype(np.float32)))
        L[k] = np.bincount(cores[k][0], minlength=4 * W)

    LMAX = np.maximum(L.max(axis=0), (L.max(axis=0) > 0).astype(np.int64))
    assert LMAX.max() <= MAX_SEG, f"segment too large: {LMAX.max()}"

    cols16 = (LMAX + 15) // 16
    tiles = (LMAX + 127) // 128
    seg_off16 = np.zeros(4 * W + 1, np.int64)
    np.cumsum(cols16, out=seg_off16[1:])
    seg_offt = np.zeros(4 * W + 1, np.int64)
    np.cumsum(tiles, out=seg_offt[1:])

    per_core = []
    for k in range(NCORES):
        key_s, idx_s, cl_s = cores[k]
        idx_arr = np.full(int(seg_off16[-1]) * 16, -1, np.int16)
        colv_arr = np.full(int(seg_offt[-1]) * 128, -1.0, np.float32)
        starts_src = np.zeros(4 * W + 1, np.int64)
        np.cumsum(L[k], out=starts_src[1:])
        seg_of = np.repeat(np.arange(4 * W), L[k])
        off_in = np.arange(len(idx_s)) - starts_src[seg_of]
        idx_arr[seg_off16[seg_of] * 16 + off_in] = idx_s.astype(np.int16)
        colv_arr[seg_offt[seg_of] * 128 + off_in] = cl_s
        # pad [L_k, lmax) of each segment with cycled copies of its own real
        # indices (S zeroes them) -- spreads pad reads across HBM instead of
        # hammering row 0
        for sgi in np.nonzero(LMAX > L[k])[0]:
            b = int(seg_off16[sgi]) * 16
            lk = int(L[k][sgi])
            lm = int(LMAX[sgi])
            if lk > 0:
                idx_arr[b + lk:b + lm] = np.resize(idx_arr[b:b + lk], lm - lk)
            else:
                idx_arr[b + lk:b + lm] = 0
        per_core.append((idx_arr, colv_arr))

    def wrap_idx(arr):
        a16 = arr.reshape(-1, 16).T
        return np.ascontiguousarray(np.tile(a16, (8, 1)))

    in_maps = []
    for k in range(NCORES):
        idx_arr, colv_arr = per_core[k]
        xs = np.zeros((PADN, F_IN), np.float32)
        xs[:SHARD] = x[k * SHARD:(k + 1) * SHARD]
        xT = np.ascontiguousarray(xs.T).astype(bf16)
        dvk = np.zeros(PADN, np.float32)
        dvk[:SHARD] = dinv[k * SHARD:(k + 1) * SHARD]
        im = {
            "xT": xT,
            "dinv_w": np.ascontiguousarray(dvk.reshape(W, 128).T),
            "lin1T": np.ascontiguousarray(lin1_w.T).astype(bf16),
            "phi1T": np.ascontiguousarray(phi1_w.T).astype(bf16),
            "aw1T": np.ascontiguousarray(
                (W1 - W1.T - GAMMA * np.eye(HID, dtype=np.float32)).T).astype(bf16),
            "lin2T": np.ascontiguousarray(lin2_w.T).astype(bf16),
            "phi2T": np.ascontiguousarray(phi2_w.T).astype(bf16),
            "aw2T": np.ascontiguousarray(
                (W2 - W2.T - GAMMA * np.eye(C, dtype=np.float32)).T).astype(bf16),
        }
        for q in range(4):
            s16 = slice(int(seg_off16[q * W]) * 16, int(seg_off16[(q + 1) * W]) * 16)
            st = slice(int(seg_offt[q * W]) * 128, int(seg_offt[(q + 1) * W]) * 128)
            im[f"idx{q}"] = wrap_idx(idx_arr[s16])
            im[f"colv{q}"] = np.ascontiguousarray(
                colv_arr[st].reshape(-1, 128).T.astype(bf16))
        in_maps.append(im)

    biases = {
        "blin1": np.broadcast_to(lin1_b, (128, HID)).astype(np.float32).copy(),
        "bconv1": np.broadcast_to(b1, (128, HID)).astype(np.float32).copy(),
        "blin2": np.broadcast_to(lin2_b, (128, C)).astype(np.float32).copy(),
        "bconv2": np.broadcast_to(b2, (128, C)).astype(np.float32).copy(),
    }
    use_bias = {name: bool(np.any(arr)) for name, arr in biases.items()}
    for name, used in use_bias.items():
        if used:
            for im in in_maps:
                im[name] = biases[name]

    import os
    meta = {
        "LMAX": LMAX,
        "tiles": tiles, "cols16": cols16,
        "seg_off16": seg_off16, "seg_offt": seg_offt,
        "use_bias": use_bias,
        "phases": int(os.environ.get("KERNEL_PHASES", "5")),
    }
    return in_maps, meta


def _build_graph(meta):
    import concourse.bass as bass
    import concourse.mybir as mybir
    import concourse.tile as tile
    from concourse import bacc
    from concourse.masks import make_identity
    from contextlib import ExitStack

    dt = mybir.dt
    Alu = mybir.AluOpType
    Act = mybir.ActivationFunctionType
    LMAX = meta["LMAX"]
    tiles = meta["tiles"]
    cols16 = meta["cols16"]
    seg_off16 = meta["seg_off16"]
    seg_offt = meta["seg_offt"]
    use_bias = meta["use_bias"]
    phases = meta.get("phases", 5)

    nc = bacc.Bacc("TRN2", target_bir_lowering=False, num_swdge_queues=4)

    xT = nc.declare_dram_parameter("xT", [F_IN, PADN], dt.bfloat16, isOutput=False)
    dinv_w = nc.declare_dram_parameter("dinv_w", [128, W], dt.float32, isOutput=False)
    lin1T = nc.declare_dram_parameter("lin1T", [F_IN, HID], dt.bfloat16, isOutput=False)
    phi1T = nc.declare_dram_parameter("phi1T", [HID, HID], dt.bfloat16, isOutput=False)
    aw1T = nc.declare_dram_parameter("aw1T", [HID, HID], dt.bfloat16, isOutput=False)
    lin2T = nc.declare_dram_parameter("lin2T", [HID, C], dt.bfloat16, isOutput=False)
    phi2T = nc.declare_dram_parameter("phi2T", [C, C], dt.bfloat16, isOutput=False)
    aw2T = nc.declare_dram_parameter("aw2T", [C, C], dt.bfloat16, isOutput=False)
    idx_p, colv_p = [], []
    for q in range(4):
        n16 = int(seg_off16[(q + 1) * W] - seg_off16[q * W])
        nt = int(seg_offt[(q + 1) * W] - seg_offt[q * W])
        idx_p.append(nc.declare_dram_parameter(f"idx{q}", [128, n16], dt.int16,
                                               isOutput=False))
        colv_p.append(nc.declare_dram_parameter(f"colv{q}", [128, nt],
                                                dt.bfloat16, isOutput=False))
    bias_p = {}
    for name, shape in [("blin1", [128, HID]), ("bconv1", [128, HID]),
                        ("blin2", [128, C]), ("bconv2", [128, C])]:
        if use_bias[name]:
            bias_p[name] = nc.declare_dram_parameter(name, shape, dt.float32,
                                                     isOutput=False)
    out_p = nc.declare_dram_parameter("out", [PADN, C], dt.float32, isOutput=True)

    t1q_in = [nc.dram_tensor(f"t1in{q}", [QROWS[q], HID], dt.bfloat16)
              for q in range(4)]
    t1q_tab = [nc.dram_tensor(f"t1tab{q}", [NCORES * QROWS[q], HID], dt.bfloat16,
                              addr_space="Shared") for q in range(4)]
    # t2 rows padded to 128 bf16 (=256B) so phase F gathers the same way as
    # phase C (no f32 repack copy, no cast of the gathered rhs)
    t2q_in = [nc.dram_tensor(f"t2in{q}", [QROWS[q], 128], dt.bfloat16)
              for q in range(4)]
    t2q_tab = [nc.dram_tensor(f"t2tab{q}", [NCORES * QROWS[q], 128], dt.bfloat16,
                              addr_space="Shared") for q in range(4)]

    rg = [list(range(NCORES))]

    with tile.TileContext(nc) as tc, ExitStack() as top:
        const = top.enter_context(tc.tile_pool(name="const", bufs=1))
        h2_pool = top.enter_context(tc.tile_pool(name="h2", bufs=1))
        agg2_pool = top.enter_context(tc.tile_pool(name="agg2", bufs=1))
        tmp_pool = top.enter_context(tc.tile_pool(name="tmp", bufs=3))

        lin1T_sb = const.tile([128, 2, HID], dt.bfloat16)
        nc.sync.dma_start(lin1T_sb[:], lin1T[:].rearrange("(t p) j -> p t j", p=128))
        phi1T_sb = const.tile([128, HID], dt.bfloat16)
        nc.sync.dma_start(phi1T_sb[:], phi1T[:])
        aw1T_sb = const.tile([128, HID], dt.bfloat16)
        nc.sync.dma_start(aw1T_sb[:], aw1T[:])
        lin2T_sb = const.tile([128, C], dt.bfloat16)
        nc.sync.dma_start(lin2T_sb[:], lin2T[:])
        phi2T_sb = const.tile([128, C], dt.bfloat16)
        aw2T_sb = const.tile([128, C], dt.bfloat16)
        for r in range(4):
            nc.sync.dma_start(phi2T_sb[r * C:(r + 1) * C, :], phi2T[:])
            nc.sync.dma_start(aw2T_sb[r * C:(r + 1) * C, :], aw2T[:])
        dinv_sb = const.tile([128, W], dt.float32)
        nc.sync.dma_start(dinv_sb[:], dinv_w[:])
        bias_sb = {}
        for name, p in bias_p.items():
            t = const.tile(list(p.shape), dt.float32)
            nc.sync.dma_start(t[:], p[:])
            bias_sb[name] = t

        iota_i = const.tile([128, 128], dt.int32)
        nc.gpsimd.iota(iota_i[:], pattern=[[1, 128]], base=0, channel_multiplier=0)
        iota_bf = const.tile([128, 128], dt.bfloat16)
        nc.vector.tensor_copy(iota_bf[:], iota_i[:])
        ident = const.tile([128, 128], dt.bfloat16)
        make_identity(nc, ident[:])

        h2 = h2_pool.tile([128, W, C], dt.float32)
        agg2 = agg2_pool.tile([128, W, C], dt.float32)
        h2T_all = h2_pool.tile([128, (W + 1) // 2, 128], dt.bfloat16,
                               tag="h2T_all")

        MAXTILES = int(tiles.max()) if len(tiles) else 1

        def aggregate(table, fw, pools, q, post_cb=None):
            gp, sp, ip, cp, psX, agg_t = pools
            n16_0 = int(seg_off16[q * W])
            nt_0 = int(seg_offt[q * W])
            n16 = int(seg_off16[(q + 1) * W]) - n16_0
            ntq = int(seg_offt[(q + 1) * W]) - nt_0
            idx_sb = ip.tile([128, n16], dt.int16, tag="idx")
            nc.sync.dma_start(idx_sb[:], idx_p[q][:])
            colv_sb = cp.tile([128, ntq], dt.bfloat16, tag="colv")
            nc.sync.dma_start(colv_sb[:], colv_p[q][:])
            for w in range(W):
                s = q * W + w
                nt = int(tiles[s])
                if nt == 0:
                    if post_cb is not None:
                        post_cb(w)
                    continue
                lmax = int(LMAX[s])
                o16 = int(seg_off16[s]) - n16_0
                ot = int(seg_offt[s]) - nt_0
                g = gp.tile([128, nt, HID], dt.bfloat16, tag="g")
                nc.gpsimd.dma_gather(
                    g[:], table[q][:], idx_sb[:, o16:o16 + int(cols16[s])],
                    lmax, lmax, 128, queue_num=w % 4)
                S = sp.tile([128, nt, 128], dt.bfloat16, tag="S")
                nc.vector.tensor_tensor(
                    S[:],
                    iota_bf[:].unsqueeze(1).broadcast_to([128, nt, 128]),
                    colv_sb[:, ot:ot + nt].unsqueeze(2)
                        .broadcast_to([128, nt, 128]),
                    op=Alu.is_equal)
                pseg = psX.tile([128, fw], dt.float32, tag="pseg")
                for j in range(nt):
                    nc.tensor.matmul(pseg[:], S[:, j, :], g[:, j, 0:fw],
                                     start=(j == 0), stop=(j == nt - 1))
                nc.vector.tensor_tensor(agg_t[:, w, :], agg_t[:, w, :], pseg[:],
                                        op=Alu.add)
                if post_cb is not None:
                    post_cb(w)

        with tc.tile_pool(name="h1cf", bufs=1) as h1_pool, \
             tc.tile_pool(name="agg", bufs=1) as agg_pool, \
             ExitStack() as stC, ExitStack() as stA, ExitStack() as stD:
            h1 = h1_pool.tile([128, W, HID], dt.bfloat16)
            h1T_all = h1_pool.tile([128, W, HID], dt.bfloat16, tag="h1T_all")
            agg = agg_pool.tile([128, W, HID], dt.float32)

            gp = stC.enter_context(tc.tile_pool(name="gp", bufs=8))
            sp = stC.enter_context(tc.tile_pool(name="sp", bufs=4))
            ip = stC.enter_context(tc.tile_pool(name="ip", bufs=2))
            cp = stC.enter_context(tc.tile_pool(name="cp", bufs=2))
            psC = stC.enter_context(tc.tile_pool(name="psC", bufs=3,
                                                 space="PSUM"))
            xqp = stA.enter_context(tc.tile_pool(name="xq", bufs=1))
            t1qbp = stA.enter_context(tc.tile_pool(name="t1qb", bufs=2))
            psA = stA.enter_context(tc.tile_pool(name="psA", bufs=1,
                                                 space="PSUM"))
            psAT = stA.enter_context(tc.tile_pool(name="psAT", bufs=1,
                                                  space="PSUM"))

            nc.vector.memset(agg[:], 0.0)
            nc.vector.memset(h2[:], 0.0)
            nc.vector.memset(agg2[:], 0.0)
            for i in range(8):
                z = gp.tile([128, MAXTILES, HID], dt.bfloat16, tag="g")
                nc.vector.memset(z[:], 0.0)

            # ===== Phase A: h1, h1T, T1 per quarter + t1 AllGather =====
            for q in range(4):
                c0 = QWSTART[q] * 128
                cw = QW[q] * 128
                xq_sb = xqp.tile([128, 2, cw], dt.bfloat16, tag="xq")
                nc.sync.dma_start(
                    xq_sb[:],
                    xT[:, c0:c0 + cw].rearrange("(t p) c -> p t c", p=128))
                t1qb = t1qbp.tile([128, QW[q], HID], dt.bfloat16, tag="t1qb")
                for wi in range(QW[q]):
                    w = QWSTART[q] + wi
                    ph = psA.tile([128, HID], dt.float32, tag="ph")
                    nc.tensor.matmul(ph[:], xq_sb[:, 0, wi * 128:(wi + 1) * 128],
                                     lin1T_sb[:, 0, :], start=True, stop=False)
                    nc.tensor.matmul(ph[:], xq_sb[:, 1, wi * 128:(wi + 1) * 128],
                                     lin1T_sb[:, 1, :], start=False, stop=True)
                    if "blin1" in bias_sb:
                        t = tmp_pool.tile([128, HID], dt.float32, tag="tA")
                        nc.vector.tensor_tensor(t[:], ph[:], bias_sb["blin1"][:],
                                                op=Alu.add)
                        nc.scalar.activation(h1[:, w, :], t[:], Act.Relu)
                    else:
                        nc.scalar.activation(h1[:, w, :], ph[:], Act.Relu)
                    pt = psAT.tile([128, 128], dt.bfloat16, tag="pt")
                    nc.tensor.transpose(pt[:], h1[:, w, :], ident[:])
                    nc.scalar.copy(h1T_all[:, w, :], pt[:])
                    pT = psA.tile([128, HID], dt.float32, tag="pT1")
                    nc.tensor.matmul(pT[:], h1T_all[:, w, :], phi1T_sb[:],
                                     start=True, stop=True)
                    nc.scalar.activation(t1qb[:, wi, :], pT[:], Act.Copy,
                                         scale=dinv_sb[:, w:w + 1])
                nc.vector.tensor_tensor(
                    agg[:, QWSTART[q]:QWSTART[q] + QW[q], :],
                    agg[:, QWSTART[q]:QWSTART[q] + QW[q], :], t1qb[:],
                    op=Alu.add)
                nc.sync.dma_start(
                    t1q_in[q][:].rearrange("(w p) f -> p w f", p=128), t1qb[:])
                nc.gpsimd.collective_compute(
                    "AllGather", Alu.bypass, replica_groups=rg,
                    ins=[t1q_in[q][:].opt()], outs=[t1q_tab[q][:].opt()])

            # ===== Phase C =====
            if phases >= 2:
                for q in range(4):
                    aggregate(t1q_tab, HID, (gp, sp, ip, cp, psC, agg), q)
            stA.close()

            # ===== Phases D + F + G (quarter-interleaved) =====
            if phases >= 3:
                t2qbp = stD.enter_context(tc.tile_pool(name="t2qb", bufs=2))
                dstg = stD.enter_context(tc.tile_pool(name="dstg", bufs=1))
                psD = stD.enter_context(tc.tile_pool(name="psD", bufs=1,
                                                     space="PSUM"))
                psDt = stD.enter_context(tc.tile_pool(name="psDt", bufs=1,
                                                      space="PSUM"))
                psDs = stD.enter_context(tc.tile_pool(name="psDs", bufs=1,
                                                      space="PSUM"))

                def d_quarter(Q):
                    qw = QW[Q]
                    w0 = QWSTART[Q]
                    t2qb = t2qbp.tile([128, qw, 128], dt.bfloat16, tag="t2qb")
                    nc.vector.memset(t2qb[:], 0.0)
                    h1p_all = dstg.tile([128, 25, HID], dt.bfloat16,
                                        tag="h1p_all")
                    h2b_all = dstg.tile([128, 25, C], dt.bfloat16,
                                        tag="h2b_all")
                    for i in range(qw):
                        w = w0 + i
                        paw = psD.tile([128, HID], dt.float32, tag="paw")
                        nc.tensor.matmul(paw[:], h1T_all[:, w, :], aw1T_sb[:],
                                         start=True, stop=True)
                        pre = tmp_pool.tile([128, HID], dt.float32, tag="pre")
                        nc.vector.scalar_tensor_tensor(
                            pre[:], agg[:, w, :], dinv_sb[:, w:w + 1], paw[:],
                            op0=Alu.mult, op1=Alu.add)
                        if "bconv1" in bias_sb:
                            nc.vector.tensor_tensor(
                                pre[:], pre[:], bias_sb["bconv1"][:], op=Alu.add)
                        th = tmp_pool.tile([128, HID], dt.float32, tag="th")
                        nc.scalar.activation(th[:], pre[:], Act.Tanh)
                        nc.vector.scalar_tensor_tensor(
                            h1p_all[:, i, :], th[:], 0.1, h1[:, w, :],
                            op0=Alu.mult, op1=Alu.add)
                    for i in range(qw):
                        w = w0 + i
                        pt2 = psDt.tile([128, 128], dt.bfloat16, tag="ptD")
                        nc.tensor.transpose(pt2[:], h1p_all[:, i, :], ident[:])
                        h1pT = tmp_pool.tile([128, 128], dt.bfloat16,
                                             tag="h1pT")
                        nc.scalar.copy(h1pT[:], pt2[:])
                        ph2 = psDs.tile([128, C], dt.float32, tag="psD2")
                        nc.tensor.matmul(ph2[:], h1pT[:], lin2T_sb[:],
                                         start=True, stop=True)
                        if "blin2" in bias_sb:
                            nc.vector.tensor_tensor(
                                h2[:, w, :], ph2[:], bias_sb["blin2"][:],
                                op=Alu.add)
                        else:
                            nc.scalar.copy(h2[:, w, :], ph2[:])
                        nc.vector.tensor_copy(h2b_all[:, i, :], h2[:, w, :])
                    for i in range(qw):
                        w = w0 + i
                        p0 = (w % 2) * 64
                        pt3 = psDt.tile([C, 128], dt.bfloat16, tag="pt3")
                        nc.tensor.transpose(pt3[:], h2b_all[:, i, :], ident[:])
                        nc.scalar.copy(h2T_all[p0:p0 + C, w // 2, :], pt3[:])
                        pT2 = psDs.tile([128, C], dt.float32, tag="psD2")
                        nc.tensor.matmul(pT2[:], h2T_all[p0:p0 + C, w // 2, :],
                                         phi2T_sb[p0:p0 + C, :],
                                         start=True, stop=True)
                        nc.scalar.activation(t2qb[:, i, 0:C], pT2[:], Act.Copy,
                                             scale=dinv_sb[:, w:w + 1])
                    nc.vector.tensor_tensor(
                        agg2[:, w0:w0 + qw, :], agg2[:, w0:w0 + qw, :],
                        t2qb[:, :, 0:C], op=Alu.add)
                    nc.sync.dma_start(
                        t2q_in[Q][:].rearrange("(w p) f -> p w f", p=128),
                        t2qb[:])
                    nc.gpsimd.collective_compute(
                        "AllGather", Alu.bypass, replica_groups=rg,
                        ins=[t2q_in[Q][:].opt()], outs=[t2q_tab[Q][:].opt()])

                GB = 4

                def g_group(w0, gw):
                    a1 = tmp_pool.tile([128, GB, C], dt.float32, tag="a1g")
                    nc.vector.tensor_tensor(
                        a1[:, 0:gw, :], agg2[:, w0:w0 + gw, :],
                        dinv_sb[:, w0:w0 + gw].unsqueeze(2)
                            .broadcast_to([128, gw, C]),
                        op=Alu.mult)
                    pre = tmp_pool.tile([128, GB, C], dt.float32, tag="preg")
                    for wi in range(gw):
                        w = w0 + wi
                        p0 = (w % 2) * 64
                        pawt = psD.tile([128, C], dt.float32, tag="pawt")
                        nc.tensor.matmul(pawt[:],
                                         h2T_all[p0:p0 + C, w // 2, :],
                                         aw2T_sb[p0:p0 + C, :],
                                         start=True, stop=True)
                        nc.vector.tensor_tensor(
                            pre[:, wi, :], a1[:, wi, :], pawt[:], op=Alu.add)
                    if "bconv2" in bias_sb:
                        nc.vector.tensor_tensor(
                            pre[:, 0:gw, :], pre[:, 0:gw, :],
                            bias_sb["bconv2"][:].unsqueeze(1)
                                .broadcast_to([128, gw, C]),
                            op=Alu.add)
                    th = tmp_pool.tile([128, GB, C], dt.float32, tag="thg")
                    nc.scalar.activation(th[:, 0:gw, :], pre[:, 0:gw, :],
                                         Act.Tanh)
                    h2p = tmp_pool.tile([128, GB, C], dt.float32, tag="h2pg")
                    nc.vector.scalar_tensor_tensor(
                        h2p[:, 0:gw, :], th[:, 0:gw, :], 0.1,
                        h2[:, w0:w0 + gw, :], op0=Alu.mult, op1=Alu.add)
                    negmax = tmp_pool.tile([128, GB, 1], dt.float32, tag="nmg")
                    nc.vector.tensor_reduce(negmax[:, 0:gw, :], h2p[:, 0:gw, :],
                                            axis=mybir.AxisListType.X,
                                            op=Alu.max, negate=True)
                    sub = tmp_pool.tile([128, GB, C], dt.float32, tag="subg")
                    nc.vector.tensor_tensor(
                        sub[:, 0:gw, :], h2p[:, 0:gw, :],
                        negmax[:, 0:gw, :].broadcast_to([128, gw, C]),
                        op=Alu.add)
                    e = tmp_pool.tile([128, GB, C], dt.float32, tag="eg")
                    nc.scalar.activation(e[:, 0:gw, :], sub[:, 0:gw, :], Act.Exp)
                    ssum = tmp_pool.tile([128, GB, 1], dt.float32, tag="ssg")
                    nc.vector.tensor_reduce(ssum[:, 0:gw, :], e[:, 0:gw, :],
                                            axis=mybir.AxisListType.X,
                                            op=Alu.add)
                    lse = tmp_pool.tile([128, GB, 1], dt.float32, tag="lseg")
                    nc.scalar.activation(lse[:, 0:gw, :], ssum[:, 0:gw, :],
                                         Act.Ln)
                    nc.vector.tensor_tensor(
                        agg2[:, w0:w0 + gw, :], sub[:, 0:gw, :],
                        lse[:, 0:gw, :].broadcast_to([128, gw, C]),
                        op=Alu.subtract)

                def g_step(w):
                    if phases < 5:
                        return
                    if (w + 1) % GB == 0 or w == W - 1:
                        w0 = (w // GB) * GB
                        g_group(w0, w - w0 + 1)

                for q in range(4):
                    d_quarter(q)
                    if phases >= 4:
                        aggregate(t2q_tab, C, (gp, sp, ip, cp, psC, agg2), q,
                                  post_cb=g_step if q == 3 else None)

        nc.sync.dma_start(out_p[:].rearrange("(w p) c -> p w c", p=128), agg2[:])

    nc.compile()
    return nc


def kernel(**inputs):
    from concourse.bass_utils import run_bass_kernel_spmd

    inp = {k: np.asarray(v) for k, v in inputs.items()}
    in_maps, meta = _host_prep(**inp)

    key = ("graph", tuple(meta["LMAX"].tolist()),
           tuple(sorted(meta["use_bias"].items())), meta["phases"])
    if key not in _CACHE:
        _CACHE[key] = _build_graph(meta)
    nc = _CACHE[key]

    import os
    trace = bool(int(os.environ.get("KERNEL_TRACE", "0")))
    res = run_bass_kernel_spmd(nc, in_maps, list(range(NCORES)), trace=trace,
                               tmpdir=os.environ.get("KERNEL_TRACE_DIR"))
    global LAST_EXEC_NS
    LAST_EXEC_NS = res.exec_time_ns

    out = np.concatenate([res.results[k]["out"][:SHARD] for k in range(NCORES)], 0)
    return out.astype(np.float32)


LAST_EXEC_NS = None

